# revision 6
# baseline (speedup 1.0000x reference)
"""Trainium2 Bass kernel for nn_AGCnet — 8-core batch-parallel.

Reference structure (B=16, C=64, H=W=256):
  x0  = AdaptiveAvgPool2d((2,2))(x)                      [B,C,2,2]
  x0  = conv3x3(x0, w1, pad 1)                           [B,C,2,2]
  x1  = conv1x1(x0, w2, stride 2, pad 1)                 [B,C,2,2]
  x1  = (x1 - x1.min()) / (x1.max() - x1.min()) * 2
  x4  = (x - x.min()) / (x.max() - x.min())
  x44 = per-quadrant exposure adjust of x4 with gammas from x1
  y   = x + (x4 * (x.max()-x.min()) + x.min())

Key algebraic reductions baked in here:
  * The stride-2/pad-1 1x1 conv samples the zero padding at 3 of its 4
    output positions, so x1[:,:,0,0] = x1[:,:,0,1] = x1[:,:,1,0] = 0 and
    only x1[:,:,1,1] = w2 @ (conv3x3 output at (1,1)) carries data.
  * The conv3x3 output at (1,1) only reads taps (kh,kw) in {0,1}^2, i.e.
    v[b,d] = sum_{o} w2[d,o] * sum_{c,i,j} pool[b,c,i,j] * w1[o,c,i,j].
  * The min-max rescale of x1 is invariant to positive scaling, so the
    /16384 pooling normalization is dropped (v is 16384x the true value).

Per core (2 batches): partition p = b*64 + c; two streaming passes over x.
Pass 1: per-(b,c) quadrant sums (ScalarE accumulate) + global min/max
(VectorE tensor_scalar reduce-accumulate).  Tiny convs as 128x128
block-diagonal matmuls.  One 4-float AllReduce(max) carries
{-xmin, xmax, -vmin, vmax} across the 8 cores.  Pass 2: normalize,
ln/exp exposure adjust (both branches blended via per-partition
scale/bias: the pow branch is killed with bias=-1e30 when gamma<1, the
log branch via a zero coefficient otherwise), and y reconstruction.
"""

import numpy as np

import concourse.bacc as bacc
import concourse.mybir as mybir
from concourse import masks, tile
from concourse.bass_utils import run_bass_kernel_spmd

F32 = mybir.dt.float32
ALU = mybir.AluOpType
AF = mybir.ActivationFunctionType
AX = mybir.AxisListType

N_CORES = 8
INV_LN2 = float(1.0 / np.log(2.0))
NEG_BIG = -1.0e30


def build_kernel(B_sh=2, C=64, H=256, W=256, r1=16, r2=8, n_cores=N_CORES,
                 finalize=True):
    P = B_sh * C
    assert P == 128
    hw = W // 2
    hh = H // 2
    T1 = H // r1
    T2 = H // r2
    assert hh % r1 == 0 and hh % r2 == 0

    nc = bacc.Bacc(None, target_bir_lowering=False, debug=False)
    x_ext = nc.declare_dram_parameter("x", [B_sh, C, H, W], F32, isOutput=False)
    w1_ext = nc.declare_dram_parameter("w1", [C, C, 3, 3], F32, isOutput=False)
    w2_ext = nc.declare_dram_parameter("w2", [C, C, 1, 1], F32, isOutput=False)
    y_ext = nc.declare_dram_parameter("y", [B_sh, C, H, W], F32, isOutput=True)
    o_ext = nc.declare_dram_parameter("x44", [B_sh, C, H, W], F32, isOutput=True)

    xv = x_ext.ap().rearrange("b c h w -> (b c) h w")
    yv = y_ext.ap().rearrange("b c h w -> (b c) h w")
    ov = o_ext.ap().rearrange("b c h w -> (b c) h w")
    groups = [list(range(n_cores))]

    with tile.TileContext(nc) as tc:
        with (
            tc.tile_pool(name="const", bufs=1) as constp,
            tc.tile_pool(name="stats", bufs=1) as statp,
            tc.tile_pool(name="psum", bufs=1, space="PSUM") as psum,
            tc.tile_pool(name="dram", bufs=1, space="DRAM") as dram,
        ):
            ident = constp.tile([P, P], F32)
            masks.make_identity(nc, ident[:])
            ones1 = constp.tile([1, P], F32)
            nc.gpsimd.memset(ones1[:], 1.0)

            w1sb = constp.tile([C, C * 9], F32)
            nc.sync.dma_start(
                out=w1sb[:], in_=w1_ext.ap().rearrange("o c kh kw -> o (c kh kw)")
            )
            w2sb = constp.tile([C, C], F32)
            nc.sync.dma_start(
                out=w2sb[:], in_=w2_ext.ap().rearrange("d o kh kw -> d (o kh kw)")
            )

            # Block-diagonal stationary weights: lhsT[(b',c), (b,o)] =
            # delta(b,b') * w1[o,c,tap] so K can stay on the (b,c) partitions.
            w1v = w1sb[:].rearrange("o (c k) -> o c k", k=9)
            w1blks = []
            for i, j in [(0, 0), (0, 1), (1, 0), (1, 1)]:
                tap = i * 3 + j
                trp = psum.tile([C, C], F32)
                nc.tensor.transpose(trp[:], w1v[:, :, tap], ident[0:C, 0:C])
                blk = constp.tile([P, P], F32)
                nc.vector.memset(blk[:], 0.0)
                nc.scalar.copy(out=blk[0:C, 0:C], in_=trp[:])
                nc.scalar.copy(out=blk[C:P, C:P], in_=trp[:])
                w1blks.append(blk)
            tr2 = psum.tile([C, C], F32)
            nc.tensor.transpose(tr2[:], w2sb[:], ident[0:C, 0:C])
            w2blk = constp.tile([P, P], F32)
            nc.vector.memset(w2blk[:], 0.0)
            nc.scalar.copy(out=w2blk[0:C, 0:C], in_=tr2[:])
            nc.scalar.copy(out=w2blk[C:P, C:P], in_=tr2[:])

            # ---------------- pass 1: stream x, gather stats ----------------
            minp = statp.tile([P, T1], F32)
            maxp = statp.tile([P, T1], F32)
            sl = statp.tile([P, T1], F32)
            sr = statp.tile([P, T1], F32)

            with (
                tc.tile_pool(name="p1x", bufs=3) as p1x,
                tc.tile_pool(name="p1scr", bufs=2) as p1scr,
                tc.tile_pool(name="p1ascr", bufs=2) as p1ascr,
            ):
                for t in range(T1):
                    r0 = t * r1
                    xt = p1x.tile([P, r1, W], F32)
                    nc.sync.dma_start(out=xt[:], in_=xv[:, r0 : r0 + r1, :])
                    s1 = p1scr.tile([P, r1, W], F32)
                    nc.vector.tensor_scalar(
                        out=s1[:], in0=xt[:], scalar1=1.0, scalar2=None,
                        op0=ALU.mult, op1=ALU.min, accum_out=minp[:, t : t + 1],
                    )
                    s2 = p1scr.tile([P, r1, W], F32)
                    nc.vector.tensor_scalar(
                        out=s2[:], in0=xt[:], scalar1=1.0, scalar2=None,
                        op0=ALU.mult, op1=ALU.max, accum_out=maxp[:, t : t + 1],
                    )
                    a1 = p1ascr.tile([P, r1, hw], F32)
                    nc.scalar.activation(
                        out=a1[:], in_=xt[:, :, 0:hw], func=AF.Copy,
                        accum_out=sl[:, t : t + 1],
                    )
                    a2 = p1ascr.tile([P, r1, hw], F32)
                    nc.scalar.activation(
                        out=a2[:], in_=xt[:, :, hw:W], func=AF.Copy,
                        accum_out=sr[:, t : t + 1],
                    )

            # ------------- finals + tiny convs + all-reduce ------------------
            ht = T1 // 2
            S = statp.tile([P, 4], F32)
            nc.vector.tensor_reduce(out=S[:, 0:1], in_=sl[:, 0:ht], axis=AX.X, op=ALU.add)
            nc.vector.tensor_reduce(out=S[:, 1:2], in_=sr[:, 0:ht], axis=AX.X, op=ALU.add)
            nc.vector.tensor_reduce(out=S[:, 2:3], in_=sl[:, ht:T1], axis=AX.X, op=ALU.add)
            nc.vector.tensor_reduce(out=S[:, 3:4], in_=sr[:, ht:T1], axis=AX.X, op=ALU.add)
            xminv = statp.tile([P, 1], F32)
            xmaxv = statp.tile([P, 1], F32)
            nc.vector.tensor_reduce(out=xminv[:], in_=minp[:], axis=AX.X, op=ALU.min)
            nc.vector.tensor_reduce(out=xmaxv[:], in_=maxp[:], axis=AX.X, op=ALU.max)

            qp = psum.tile([P, 1], F32)
            for k in range(4):
                nc.tensor.matmul(
                    qp[:], lhsT=w1blks[k][:], rhs=S[:, k : k + 1],
                    start=(k == 0), stop=(k == 3),
                )
            qsb = statp.tile([P, 1], F32)
            nc.scalar.copy(out=qsb[:], in_=qp[:])
            vp = psum.tile([P, 1], F32)
            nc.tensor.matmul(vp[:], lhsT=w2blk[:], rhs=qsb[:], start=True, stop=True)
            vsb = statp.tile([P, 1], F32)
            nc.scalar.copy(out=vsb[:], in_=vp[:])

            # pack [-xmin, xmax, -v, v] -> [4,128] -> rowwise max -> [4,1]
            pk = statp.tile([P, 4], F32)
            nc.vector.tensor_scalar(out=pk[:, 0:1], in0=xminv[:], scalar1=-1.0,
                                    scalar2=None, op0=ALU.mult)
            nc.vector.tensor_copy(out=pk[:, 1:2], in_=xmaxv[:])
            nc.vector.tensor_scalar(out=pk[:, 2:3], in0=vsb[:], scalar1=-1.0,
                                    scalar2=None, op0=ALU.mult)
            nc.vector.tensor_copy(out=pk[:, 3:4], in_=vsb[:])
            pkt = psum.tile([4, P], F32)
            nc.tensor.transpose(pkt[:], pk[:], ident[:])
            red4 = statp.tile([4, 1], F32)
            nc.vector.tensor_reduce(out=red4[:], in_=pkt[:], axis=AX.X, op=ALU.max)

            cc_in = dram.tile([4, 1], F32)
            cc_out = dram.tile([4, 1], F32)
            nc.gpsimd.dma_start(out=cc_in[:], in_=red4[:])
            nc.gpsimd.collective_compute(
                "AllReduce", ALU.max, replica_groups=groups,
                ins=[cc_in[:].opt()], outs=[cc_out[:].opt()],
            )
            gsb = statp.tile([1, 4], F32)
            nc.gpsimd.dma_start(out=gsb[:], in_=cc_out[:])

            # broadcast the 4 reduced scalars to all 128 partitions
            gps = psum.tile([P, 4], F32)
            nc.tensor.matmul(gps[:], lhsT=ones1[:], rhs=gsb[:], start=True, stop=True)
            G = statp.tile([P, 4], F32)  # cols: -x2, x3, -vmin_g, vmax_g
            nc.scalar.copy(out=G[:], in_=gps[:])

            def pvec(tag):
                return statp.tile([P, 1], F32, name=tag, tag=tag)

            c_x2 = pvec("c_x2")
            nc.vector.tensor_scalar(out=c_x2[:], in0=G[:, 0:1], scalar1=-1.0,
                                    scalar2=None, op0=ALU.mult)
            c_r = pvec("c_r")
            nc.vector.tensor_tensor(out=c_r[:], in0=G[:, 1:2], in1=G[:, 0:1], op=ALU.add)
            c_invr = pvec("c_invr")
            nc.vector.reciprocal(out=c_invr[:], in_=c_r[:])
            c_negm0 = pvec("c_negm0")  # -m0 = max(0, -vmin_g)
            nc.vector.tensor_scalar(out=c_negm0[:], in0=G[:, 2:3], scalar1=0.0,
                                    scalar2=None, op0=ALU.max)
            c_M0 = pvec("c_M0")
            nc.vector.tensor_scalar(out=c_M0[:], in0=G[:, 3:4], scalar1=0.0,
                                    scalar2=None, op0=ALU.max)
            c_rng = pvec("c_rng")
            nc.vector.tensor_tensor(out=c_rng[:], in0=c_M0[:], in1=c_negm0[:], op=ALU.add)
            c_invg = pvec("c_invg")
            nc.vector.reciprocal(out=c_invg[:], in_=c_rng[:])
            c_tw = pvec("c_tw")
            nc.vector.tensor_scalar(out=c_tw[:], in0=c_invg[:], scalar1=2.0,
                                    scalar2=None, op0=ALU.mult)
            c_gabr = pvec("c_gabr")  # (v - m0) * 2/(M0-m0)
            nc.vector.tensor_scalar(out=c_gabr[:], in0=vsb[:], scalar1=c_negm0[:],
                                    scalar2=c_tw[:], op0=ALU.add, op1=ALU.mult)
            c_ga0 = pvec("c_ga0")  # (0 - m0) * 2/(M0-m0)
            nc.vector.tensor_tensor(out=c_ga0[:], in0=c_negm0[:], in1=c_tw[:], op=ALU.mult)
            c_mbr = pvec("c_mbr")
            nc.vector.tensor_scalar(out=c_mbr[:], in0=c_gabr[:], scalar1=1.0,
                                    scalar2=None, op0=ALU.is_lt)
            c_m0m = pvec("c_m0m")
            nc.vector.tensor_scalar(out=c_m0m[:], in0=c_ga0[:], scalar1=1.0,
                                    scalar2=None, op0=ALU.is_lt)
            c_lcbr = pvec("c_lcbr")  # mask * gamma / ln2
            nc.vector.scalar_tensor_tensor(out=c_lcbr[:], in0=c_gabr[:], scalar=INV_LN2,
                                           in1=c_mbr[:], op0=ALU.mult, op1=ALU.mult)
            c_lc0 = pvec("c_lc0")
            nc.vector.scalar_tensor_tensor(out=c_lc0[:], in0=c_ga0[:], scalar=INV_LN2,
                                           in1=c_m0m[:], op0=ALU.mult, op1=ALU.mult)
            c_pbbr = pvec("c_pbbr")  # -1e30 where log branch, else 0
            nc.vector.tensor_scalar(out=c_pbbr[:], in0=c_mbr[:], scalar1=NEG_BIG,
                                    scalar2=None, op0=ALU.mult)
            c_pb0 = pvec("c_pb0")
            nc.vector.tensor_scalar(out=c_pb0[:], in0=c_m0m[:], scalar1=NEG_BIG,
                                    scalar2=None, op0=ALU.mult)

            # ---------------- pass 2: stream x, emit y and x44 ----------------
            with (
                tc.tile_pool(name="p2x", bufs=3) as p2x,
                tc.tile_pool(name="p2t", bufs=2) as p2t,
                tc.tile_pool(name="p2a", bufs=3) as p2a,
                tc.tile_pool(name="p2u", bufs=2) as p2u,
                tc.tile_pool(name="p2g", bufs=3) as p2g,
            ):
                for t in range(T2):
                    r0 = t * r2
                    top = (r0 + r2) <= hh
                    xt = p2x.tile([P, r2, W], F32)
                    nc.sync.dma_start(out=xt[:], in_=xv[:, r0 : r0 + r2, :])
                    tt = p2t.tile([P, r2, W], F32)  # t = (x - x2) / r
                    nc.vector.tensor_scalar(out=tt[:], in0=xt[:], scalar1=G[:, 0:1],
                                            scalar2=c_invr[:], op0=ALU.add, op1=ALU.mult)
                    a_ = p2a.tile([P, r2, W], F32)  # ln(1 + t)
                    nc.scalar.activation(out=a_[:], in_=tt[:], func=AF.Ln, bias=1.0)
                    u_ = p2u.tile([P, r2, W], F32)  # ln(t)
                    nc.scalar.activation(out=u_[:], in_=tt[:], func=AF.Ln)
                    g_ = p2g.tile([P, r2, W], F32)  # exp(ga*ln t + pbias)
                    if top:
                        nc.scalar.activation(out=g_[:], in_=u_[:], func=AF.Exp,
                                             scale=c_ga0[:], bias=c_pb0[:])
                        nc.vector.scalar_tensor_tensor(
                            out=g_[:], in0=a_[:], scalar=c_lc0[:], in1=g_[:],
                            op0=ALU.mult, op1=ALU.add)
                    else:
                        nc.scalar.activation(out=g_[:, :, 0:hw], in_=u_[:, :, 0:hw],
                                             func=AF.Exp, scale=c_ga0[:], bias=c_pb0[:])
                        nc.scalar.activation(out=g_[:, :, hw:W], in_=u_[:, :, hw:W],
                                             func=AF.Exp, scale=c_gabr[:], bias=c_pbbr[:])
                        nc.vector.scalar_tensor_tensor(
                            out=g_[:, :, 0:hw], in0=a_[:, :, 0:hw], scalar=c_lc0[:],
                            in1=g_[:, :, 0:hw], op0=ALU.mult, op1=ALU.add)
                        nc.vector.scalar_tensor_tensor(
                            out=g_[:, :, hw:W], in0=a_[:, :, hw:W], scalar=c_lcbr[:],
                            in1=g_[:, :, hw:W], op0=ALU.mult, op1=ALU.add)
                    # x5 = t*r + x2 (in place over t), y = x + x5 (into a_)
                    nc.vector.tensor_scalar(out=tt[:], in0=tt[:], scalar1=c_r[:],
                                            scalar2=c_x2[:], op0=ALU.mult, op1=ALU.add)
                    nc.vector.tensor_tensor(out=a_[:], in0=xt[:], in1=tt[:], op=ALU.add)
                    nc.sync.dma_start(out=yv[:, r0 : r0 + r2, :], in_=a_[:])
                    nc.sync.dma_start(out=ov[:, r0 : r0 + r2, :], in_=g_[:])
    if finalize:
        nc.finalize()
    return nc


_NC_CACHE = {}


def _get_nc(**kw):
    key = tuple(sorted(kw.items()))
    if key not in _NC_CACHE:
        _NC_CACHE[key] = build_kernel(**kw)
    return _NC_CACHE[key]


def kernel(x, w1, w2):
    x = np.ascontiguousarray(x, dtype=np.float32)
    w1 = np.ascontiguousarray(w1, dtype=np.float32)
    w2 = np.ascontiguousarray(w2, dtype=np.float32)
    B = x.shape[0]
    bs = B // N_CORES
    nc = _get_nc(B_sh=bs, C=x.shape[1], H=x.shape[2], W=x.shape[3])
    in_maps = [
        {"x": x[i * bs : (i + 1) * bs], "w1": w1, "w2": w2} for i in range(N_CORES)
    ]
    res = run_bass_kernel_spmd(nc, in_maps, core_ids=list(range(N_CORES)))
    y = np.concatenate([res.results[i]["y"] for i in range(N_CORES)], axis=0)
    x44 = np.concatenate([res.results[i]["x44"] for i in range(N_CORES)], axis=0)
    return y, x44


# revision 17
# speedup vs baseline: 1.1087x; 1.1087x over previous
"""Trainium2 Bass kernel for nn_AGCnet — 8-core batch-parallel.

Reference structure (B=16, C=64, H=W=256):
  x0  = AdaptiveAvgPool2d((2,2))(x)                      [B,C,2,2]
  x0  = conv3x3(x0, w1, pad 1)                           [B,C,2,2]
  x1  = conv1x1(x0, w2, stride 2, pad 1)                 [B,C,2,2]
  x1  = (x1 - x1.min()) / (x1.max() - x1.min()) * 2
  x4  = (x - x.min()) / (x.max() - x.min())
  x44 = per-quadrant exposure adjust of x4 with gammas from x1
  y   = x + (x4 * (x.max()-x.min()) + x.min())

Key algebraic reductions baked in here:
  * The stride-2/pad-1 1x1 conv samples the zero padding at 3 of its 4
    output positions, so x1[:,:,0,0] = x1[:,:,0,1] = x1[:,:,1,0] = 0 and
    only x1[:,:,1,1] = w2 @ (conv3x3 output at (1,1)) carries data.
  * The conv3x3 output at (1,1) only reads taps (kh,kw) in {0,1}^2, i.e.
    v[b,d] = sum_{o} w2[d,o] * sum_{c,i,j} pool[b,c,i,j] * w1[o,c,i,j].
  * The min-max rescale of x1 is invariant to positive scaling, so the
    /16384 pooling normalization is dropped (v is 16384x the true value).

Per core (2 batches): partition p = b*64 + c; two streaming passes over x.
Pass 1: per-(b,c) quadrant sums (ScalarE accumulate) + global min/max
(VectorE tensor_scalar reduce-accumulate).  Tiny convs as 128x128
block-diagonal matmuls.  One 4-float AllReduce(max) carries
{-xmin, xmax, -vmin, vmax} across the 8 cores.  Pass 2: normalize,
ln/exp exposure adjust (both branches blended via per-partition
scale/bias: the pow branch is killed with bias=-1e30 when gamma<1, the
log branch via a zero coefficient otherwise), and y reconstruction.
"""

import numpy as np

import concourse.bacc as bacc
import concourse.mybir as mybir
from concourse import masks, tile
from concourse.bass_utils import run_bass_kernel_spmd

F32 = mybir.dt.float32
ALU = mybir.AluOpType
AF = mybir.ActivationFunctionType
AX = mybir.AxisListType

N_CORES = 8
INV_LN2 = float(1.0 / np.log(2.0))
NEG_BIG = -1.0e30
PRE_K = 4  # pass-2 tiles prefetched before the collective

_ACT_SET = "natural_log_exp_and_others"  # holds ln+exp+copy: one table load


def _patch_act_tables():
    # The greedy table-set chooser pairs Ln with "natural_log" and Exp with
    # "exp_and_others", reloading tables (~1.3us) around every activation.
    # Every function this kernel uses lives in _ACT_SET, so blank out the
    # other sets (indices must be preserved — they are act_func_set_ids).
    if getattr(bacc, "_agc_act_patch", False):
        return
    orig = bacc.get_activation_tables

    def patched(arch):
        tabs = orig(arch)
        if not any(n == _ACT_SET for n in tabs):
            return tabs
        return {n: (fns if n == _ACT_SET else set()) for n, fns in tabs.items()}

    bacc.get_activation_tables = patched
    bacc._agc_act_patch = True


def build_kernel(B_sh=2, C=64, H=256, W=256, r1=16, r2=8, n_cores=N_CORES,
                 finalize=True):
    P = B_sh * C
    assert P == 128
    hw = W // 2
    hh = H // 2
    T1 = H // r1
    T2 = H // r2
    assert hh % r1 == 0 and hh % r2 == 0

    nc = bacc.Bacc(None, target_bir_lowering=False, debug=False)
    x_ext = nc.declare_dram_parameter("x", [B_sh, C, H, W], F32, isOutput=False)
    w1_ext = nc.declare_dram_parameter("w1", [C, C, 3, 3], F32, isOutput=False)
    w2_ext = nc.declare_dram_parameter("w2", [C, C, 1, 1], F32, isOutput=False)
    y_ext = nc.declare_dram_parameter("y", [B_sh, C, H, W], F32, isOutput=True)
    o_ext = nc.declare_dram_parameter("x44", [B_sh, C, H, W], F32, isOutput=True)

    xv = x_ext.ap().rearrange("b c h w -> (b c) h w")
    yv = y_ext.ap().rearrange("b c h w -> (b c) h w")
    ov = o_ext.ap().rearrange("b c h w -> (b c) h w")
    groups = [list(range(n_cores))]

    with tile.TileContext(nc) as tc:
        with (
            tc.tile_pool(name="const", bufs=1) as constp,
            tc.tile_pool(name="stats", bufs=1) as statp,
            tc.tile_pool(name="psum", bufs=1, space="PSUM") as psum,
            tc.tile_pool(name="dram", bufs=1, space="DRAM") as dram,
        ):
            ident = constp.tile([P, P], F32)
            masks.make_identity(nc, ident[:])
            ones1 = constp.tile([1, P], F32)
            nc.gpsimd.memset(ones1[:], 1.0)

            w1sb = constp.tile([C, C * 9], F32)
            nc.sync.dma_start(
                out=w1sb[:], in_=w1_ext.ap().rearrange("o c kh kw -> o (c kh kw)")
            )
            w2sb = constp.tile([C, C], F32)
            nc.sync.dma_start(
                out=w2sb[:], in_=w2_ext.ap().rearrange("d o kh kw -> d (o kh kw)")
            )

            # Block-diagonal stationary weights: lhsT[(b',c), (b,o)] =
            # delta(b,b') * w1[o,c,tap] so K can stay on the (b,c) partitions.
            w1v = w1sb[:].rearrange("o (c k) -> o c k", k=9)
            w1blks = []
            for i, j in [(0, 0), (0, 1), (1, 0), (1, 1)]:
                tap = i * 3 + j
                trp = psum.tile([C, C], F32)
                nc.tensor.transpose(trp[:], w1v[:, :, tap], ident[0:C, 0:C])
                blk = constp.tile([P, P], F32)
                nc.vector.memset(blk[:], 0.0)
                nc.scalar.copy(out=blk[0:C, 0:C], in_=trp[:])
                nc.scalar.copy(out=blk[C:P, C:P], in_=trp[:])
                w1blks.append(blk)
            tr2 = psum.tile([C, C], F32)
            nc.tensor.transpose(tr2[:], w2sb[:], ident[0:C, 0:C])
            w2blk = constp.tile([P, P], F32)
            nc.vector.memset(w2blk[:], 0.0)
            nc.scalar.copy(out=w2blk[0:C, 0:C], in_=tr2[:])
            nc.scalar.copy(out=w2blk[C:P, C:P], in_=tr2[:])

            # ---------------- pass 1: stream x, gather stats ----------------
            # DVE: running min via tensor_scalar reduce-accumulate (1x rate).
            # GpSimd: running elementwise max into gacc (frees DVE cycles).
            # ScalarE: left/right row sums via activation accumulate.
            minp = statp.tile([P, T1], F32)
            maxp = statp.tile([P, T1], F32)
            sl = statp.tile([P, T1], F32)
            sr = statp.tile([P, T1], F32)

            from contextlib import ExitStack

            es = ExitStack()
            p1x = es.enter_context(tc.tile_pool(name="p1x", bufs=2))
            p1scr = es.enter_context(tc.tile_pool(name="p1scr", bufs=1))
            p1ascr = es.enter_context(tc.tile_pool(name="p1ascr", bufs=1))
            p2x = es.enter_context(tc.tile_pool(name="p2x", bufs=PRE_K + 1))
            p2t = es.enter_context(tc.tile_pool(name="p2t", bufs=2))
            p2a = es.enter_context(tc.tile_pool(name="p2a", bufs=2))
            p2u = es.enter_context(tc.tile_pool(name="p2u", bufs=2))
            p2g = es.enter_context(tc.tile_pool(name="p2g", bufs=2))
            for t in range(T1):
                r0 = t * r1
                xt = p1x.tile([P, r1, W], F32)
                nc.sync.dma_start(out=xt[:], in_=xv[:, r0 : r0 + r1, :])
                s1 = p1scr.tile([P, r1, W], F32)
                nc.vector.tensor_scalar(
                    out=s1[:], in0=xt[:], scalar1=1.0, scalar2=None,
                    op0=ALU.mult, op1=ALU.min, accum_out=minp[:, t : t + 1],
                )
                s2 = p1scr.tile([P, r1, W], F32, name="p1s2", tag="p1s2")
                nc.vector.tensor_scalar(
                    out=s2[:], in0=xt[:], scalar1=1.0, scalar2=None,
                    op0=ALU.mult, op1=ALU.max, accum_out=maxp[:, t : t + 1],
                )
                a1 = p1ascr.tile([P, r1, W], F32)
                nc.scalar.activation(
                    out=a1[:, :, 0:hw], in_=xt[:, :, 0:hw], func=AF.Copy,
                    accum_out=sl[:, t : t + 1],
                )
                nc.scalar.activation(
                    out=a1[:, :, hw:W], in_=xt[:, :, hw:W], func=AF.Copy,
                    accum_out=sr[:, t : t + 1],
                )

            # prefetch the first pass-2 tiles; they only need SBUF slots, so
            # their DMAs fill the otherwise-idle collective window
            pre = []
            for t in range(min(PRE_K, T2)):
                xt = p2x.tile([P, r2, W], F32, name="p2xt", tag="p2xt")
                nc.sync.dma_start(out=xt[:], in_=xv[:, t * r2 : (t + 1) * r2, :])
                pre.append(xt)

            # ------------- finals + tiny convs + all-reduce ------------------
            ht = T1 // 2
            S = statp.tile([P, 4], F32)
            nc.vector.tensor_reduce(out=S[:, 0:1], in_=sl[:, 0:ht], axis=AX.X, op=ALU.add)
            nc.vector.tensor_reduce(out=S[:, 1:2], in_=sr[:, 0:ht], axis=AX.X, op=ALU.add)
            nc.vector.tensor_reduce(out=S[:, 2:3], in_=sl[:, ht:T1], axis=AX.X, op=ALU.add)
            nc.vector.tensor_reduce(out=S[:, 3:4], in_=sr[:, ht:T1], axis=AX.X, op=ALU.add)
            xminv = statp.tile([P, 1], F32)
            xmaxv = statp.tile([P, 1], F32)
            nc.vector.tensor_reduce(out=xminv[:], in_=minp[:], axis=AX.X, op=ALU.min)
            nc.vector.tensor_reduce(out=xmaxv[:], in_=maxp[:], axis=AX.X, op=ALU.max)

            qp = psum.tile([P, 1], F32)
            for k in range(4):
                nc.tensor.matmul(
                    qp[:], lhsT=w1blks[k][:], rhs=S[:, k : k + 1],
                    start=(k == 0), stop=(k == 3),
                )
            qsb = statp.tile([P, 1], F32)
            nc.scalar.copy(out=qsb[:], in_=qp[:])
            vp = psum.tile([P, 1], F32)
            nc.tensor.matmul(vp[:], lhsT=w2blk[:], rhs=qsb[:], start=True, stop=True)
            vsb = statp.tile([P, 1], F32)
            nc.scalar.copy(out=vsb[:], in_=vp[:])

            # pack [-xmin, xmax, -v, v] -> [4,128] -> rowwise max -> [4,1]
            pk = statp.tile([P, 4], F32)
            nc.vector.tensor_scalar(out=pk[:, 0:1], in0=xminv[:], scalar1=-1.0,
                                    scalar2=None, op0=ALU.mult)
            nc.vector.tensor_copy(out=pk[:, 1:2], in_=xmaxv[:])
            nc.vector.tensor_scalar(out=pk[:, 2:3], in0=vsb[:], scalar1=-1.0,
                                    scalar2=None, op0=ALU.mult)
            nc.vector.tensor_copy(out=pk[:, 3:4], in_=vsb[:])
            pkt = psum.tile([4, P], F32)
            nc.tensor.transpose(pkt[:], pk[:], ident[:])
            red4 = statp.tile([4, 1], F32)
            nc.vector.tensor_reduce(out=red4[:], in_=pkt[:], axis=AX.X, op=ALU.max)

            cc_in = dram.tile([4, 1], F32)
            cc_out = dram.tile([4, 1], F32)
            nc.gpsimd.dma_start(out=cc_in[:], in_=red4[:])
            nc.gpsimd.collective_compute(
                "AllReduce", ALU.max, replica_groups=groups,
                ins=[cc_in[:].opt()], outs=[cc_out[:].opt()],
            )
            gsb = statp.tile([1, 4], F32)
            nc.gpsimd.dma_start(out=gsb[:], in_=cc_out[:])

            # broadcast the 4 reduced scalars to all 128 partitions
            gps = psum.tile([P, 4], F32)
            nc.tensor.matmul(gps[:], lhsT=ones1[:], rhs=gsb[:], start=True, stop=True)
            G = statp.tile([P, 4], F32)  # cols: -x2, x3, -vmin_g, vmax_g
            nc.scalar.copy(out=G[:], in_=gps[:])

            def pvec(tag):
                return statp.tile([P, 1], F32, name=tag, tag=tag)

            c_x2 = pvec("c_x2")
            nc.vector.tensor_scalar(out=c_x2[:], in0=G[:, 0:1], scalar1=-1.0,
                                    scalar2=None, op0=ALU.mult)
            c_r = pvec("c_r")
            nc.vector.tensor_tensor(out=c_r[:], in0=G[:, 1:2], in1=G[:, 0:1], op=ALU.add)
            c_invr = pvec("c_invr")
            nc.vector.reciprocal(out=c_invr[:], in_=c_r[:])
            c_negm0 = pvec("c_negm0")  # -m0 = max(0, -vmin_g)
            nc.vector.tensor_scalar(out=c_negm0[:], in0=G[:, 2:3], scalar1=0.0,
                                    scalar2=None, op0=ALU.max)
            c_M0 = pvec("c_M0")
            nc.vector.tensor_scalar(out=c_M0[:], in0=G[:, 3:4], scalar1=0.0,
                                    scalar2=None, op0=ALU.max)
            c_rng = pvec("c_rng")
            nc.vector.tensor_tensor(out=c_rng[:], in0=c_M0[:], in1=c_negm0[:], op=ALU.add)
            c_invg = pvec("c_invg")
            nc.vector.reciprocal(out=c_invg[:], in_=c_rng[:])
            c_tw = pvec("c_tw")
            nc.vector.tensor_scalar(out=c_tw[:], in0=c_invg[:], scalar1=2.0,
                                    scalar2=None, op0=ALU.mult)
            c_gabr = pvec("c_gabr")  # (v - m0) * 2/(M0-m0)
            nc.vector.tensor_scalar(out=c_gabr[:], in0=vsb[:], scalar1=c_negm0[:],
                                    scalar2=c_tw[:], op0=ALU.add, op1=ALU.mult)
            c_ga0 = pvec("c_ga0")  # (0 - m0) * 2/(M0-m0)
            nc.vector.tensor_tensor(out=c_ga0[:], in0=c_negm0[:], in1=c_tw[:], op=ALU.mult)
            c_mbr = pvec("c_mbr")
            nc.vector.tensor_scalar(out=c_mbr[:], in0=c_gabr[:], scalar1=1.0,
                                    scalar2=None, op0=ALU.is_lt)
            c_m0m = pvec("c_m0m")
            nc.vector.tensor_scalar(out=c_m0m[:], in0=c_ga0[:], scalar1=1.0,
                                    scalar2=None, op0=ALU.is_lt)
            c_lcbr = pvec("c_lcbr")  # mask * gamma / ln2
            nc.vector.scalar_tensor_tensor(out=c_lcbr[:], in0=c_gabr[:], scalar=INV_LN2,
                                           in1=c_mbr[:], op0=ALU.mult, op1=ALU.mult)
            c_lc0 = pvec("c_lc0")
            nc.vector.scalar_tensor_tensor(out=c_lc0[:], in0=c_ga0[:], scalar=INV_LN2,
                                           in1=c_m0m[:], op0=ALU.mult, op1=ALU.mult)
            c_pbbr = pvec("c_pbbr")  # -1e30 where log branch, else 0
            nc.vector.tensor_scalar(out=c_pbbr[:], in0=c_mbr[:], scalar1=NEG_BIG,
                                    scalar2=None, op0=ALU.mult)
            c_pb0 = pvec("c_pb0")
            nc.vector.tensor_scalar(out=c_pb0[:], in0=c_m0m[:], scalar1=NEG_BIG,
                                    scalar2=None, op0=ALU.mult)

            # ---------------- pass 2: stream x, emit y and x44 ----------------
            if True:
                for t in range(T2):
                    r0 = t * r2
                    top = (r0 + r2) <= hh
                    if t < len(pre):
                        xt = pre[t]
                    else:
                        xt = p2x.tile([P, r2, W], F32, name="p2xt", tag="p2xt")
                        nc.sync.dma_start(out=xt[:], in_=xv[:, r0 : r0 + r2, :])
                    tt = p2t.tile([P, r2, W], F32)  # t = (x - x2) / r
                    nc.vector.tensor_scalar(out=tt[:], in0=xt[:], scalar1=G[:, 0:1],
                                            scalar2=c_invr[:], op0=ALU.add, op1=ALU.mult)
                    a_ = p2a.tile([P, r2, W], F32)  # ln(1 + t)
                    nc.scalar.activation(out=a_[:], in_=tt[:], func=AF.Ln, bias=1.0)
                    u_ = p2u.tile([P, r2, W], F32)  # ln(t)
                    nc.scalar.activation(out=u_[:], in_=tt[:], func=AF.Ln)
                    g_ = p2g.tile([P, r2, W], F32)  # exp(ga*ln t + pbias)
                    if top:
                        nc.scalar.activation(out=g_[:], in_=u_[:], func=AF.Exp,
                                             scale=c_ga0[:], bias=c_pb0[:])
                        nc.vector.scalar_tensor_tensor(
                            out=g_[:], in0=a_[:], scalar=c_lc0[:], in1=g_[:],
                            op0=ALU.mult, op1=ALU.add)
                    else:
                        nc.scalar.activation(out=g_[:, :, 0:hw], in_=u_[:, :, 0:hw],
                                             func=AF.Exp, scale=c_ga0[:], bias=c_pb0[:])
                        nc.scalar.activation(out=g_[:, :, hw:W], in_=u_[:, :, hw:W],
                                             func=AF.Exp, scale=c_gabr[:], bias=c_pbbr[:])
                        nc.vector.scalar_tensor_tensor(
                            out=g_[:, :, 0:hw], in0=a_[:, :, 0:hw], scalar=c_lc0[:],
                            in1=g_[:, :, 0:hw], op0=ALU.mult, op1=ALU.add)
                        nc.vector.scalar_tensor_tensor(
                            out=g_[:, :, hw:W], in0=a_[:, :, hw:W], scalar=c_lcbr[:],
                            in1=g_[:, :, hw:W], op0=ALU.mult, op1=ALU.add)
                    # x5 = t*r + x2 (in place over t), y = x + x5 (into a_)
                    nc.vector.tensor_scalar(out=tt[:], in0=tt[:], scalar1=c_r[:],
                                            scalar2=c_x2[:], op0=ALU.mult, op1=ALU.add)
                    nc.vector.tensor_tensor(out=a_[:], in0=xt[:], in1=tt[:], op=ALU.add)
                    nc.sync.dma_start(out=yv[:, r0 : r0 + r2, :], in_=a_[:])
                    nc.sync.dma_start(out=ov[:, r0 : r0 + r2, :], in_=g_[:])
            es.close()
    if finalize:
        _patch_act_tables()
        nc.finalize()
    return nc


_NC_CACHE = {}


def _get_nc(**kw):
    key = tuple(sorted(kw.items()))
    if key not in _NC_CACHE:
        _NC_CACHE[key] = build_kernel(**kw)
    return _NC_CACHE[key]


def kernel(x, w1, w2):
    x = np.ascontiguousarray(x, dtype=np.float32)
    w1 = np.ascontiguousarray(w1, dtype=np.float32)
    w2 = np.ascontiguousarray(w2, dtype=np.float32)
    B = x.shape[0]
    bs = B // N_CORES
    nc = _get_nc(B_sh=bs, C=x.shape[1], H=x.shape[2], W=x.shape[3])
    in_maps = [
        {"x": x[i * bs : (i + 1) * bs], "w1": w1, "w2": w2} for i in range(N_CORES)
    ]
    res = run_bass_kernel_spmd(nc, in_maps, core_ids=list(range(N_CORES)))
    y = np.concatenate([res.results[i]["y"] for i in range(N_CORES)], axis=0)
    x44 = np.concatenate([res.results[i]["x44"] for i in range(N_CORES)], axis=0)
    return y, x44
